# revision 3
# baseline (speedup 1.0000x reference)
"""Trainium2 Bass kernel for nn_CascadeGNN (2-layer GCN + mean/max pool + cls).

v3 — optimized for the axon PJRT per-call cost model: measured per-call time
= RPC floor + c1 * NEFF-size(~instruction count) + c2 * input-bytes, with
actual device exec being tiny. So this version:
  - uses a UNIFORM edge schedule (per-quarter column count Gq shared by all
    tiles/cores) so both conv layers are a single hardware loop (tc.For_i)
    over the 16 graph-slots -> ~10x fewer instructions than full unrolling;
  - runs are graph-aligned (RUN == TG), which kills the batch one-hot
    (mean-pool accumulates straight into a per-run PSUM column) and the
    separate batch/count inputs;
  - ships ~1.2 MB per core: fp16 xT, unreplicated int16 gather indices,
    uint8 dst rows, and one packed fp16 weight/constant tensor;
  - computes u0/u1 on the own shard and AllGathers the u tables on-device.

Math identity: with u = dis * h, a GCN layer is
    h' = relu(dis * (sum_{e: src->n} u[src] + u[n]) @ W + b)
so cores exchange only u tables and apply W post-aggregation.  Edge messages
are fetched with SWDGE dma_gather (int16 indices over <=32767-row table
quarters) and segment-summed on the TensorEngine via one-hot matrices built
on the VectorEngine from the uint8 dst-row table; PSUM accumulates; the self
term is an identity matmul from the SBUF-resident own-u slab.  Pad nodes are
zeroed via dis=0 / padmask.

The Bass program is compiled per input instance (edge schedule baked in).
"""
import numpy as np

P = 128
NCORES = 8
H = 64
D_IN = 8
GPC = 16

N = 100000
E = 1600000
G = 128
C = 2


# ----------------------------------------------------------------------------
# host-side metadata (sharding / index prep)
# ----------------------------------------------------------------------------

def build_meta(src, dst, batch):
    graph_start = np.searchsorted(batch, np.arange(G + 1))
    gsizes = (graph_start[1:] - graph_start[:-1]).astype(np.int64)
    TG = int(np.ceil(max(int(gsizes.max()), 1) / P))
    T = GPC * TG
    S_pad = T * P
    TBL = NCORES * S_pad
    NQ = int(np.ceil(TBL / 32767.0))
    QROWS = int(np.ceil(TBL / NQ / P)) * P

    # node -> padded table row (logical: local = tile*128 + partition)
    map_row = np.empty(N, np.int64)
    for g in range(G):
        k, slot = g // GPC, g % GPC
        a, b = graph_start[g], graph_start[g + 1]
        map_row[a:b] = k * S_pad + slot * TG * P + np.arange(b - a)

    deg = np.bincount(dst, minlength=N).astype(np.float64) + 1.0
    dis = (1.0 / np.sqrt(deg)).astype(np.float32)

    order = np.argsort(dst, kind="stable")
    src_s = src[order].astype(np.int64)
    dst_s = dst[order].astype(np.int64)
    # primed (partition-major) table row of the source
    sr = map_row[src_s]
    sk, sloc = sr // S_pad, sr % S_pad
    src_rowp = sk * S_pad + (sloc % P) * T + (sloc // P)
    src_q = src_rowp // QROWS
    src_rel = (src_rowp - src_q * QROWS).astype(np.int64)
    dst_row = map_row[dst_s]

    buckets = {}
    cnt = np.zeros((NCORES, T, NQ), np.int64)
    for k in range(NCORES):
        e0 = np.searchsorted(dst_row, k * S_pad)
        e1 = np.searchsorted(dst_row, (k + 1) * S_pad)
        loc = dst_row[e0:e1] - k * S_pad
        tq = loc // P
        t_start = e0 + np.searchsorted(tq, np.arange(T + 1))
        for t in range(T):
            a, b = t_start[t], t_start[t + 1]
            q_e = src_q[a:b]
            loc_t = loc[a - e0:b - e0] - t * P
            for q in range(NQ):
                m = q_e == q
                buckets[(k, t, q)] = (src_rel[a:b][m], loc_t[m])
                cnt[k, t, q] = int(m.sum())

    # uniform schedule: per-quarter column count shared by all tiles/cores
    Gq = [int(x) for x in (-(-cnt // P)).max(axis=(0, 1))]
    S = int(sum(Gq))
    soff = [0]
    for q in range(NQ):
        soff.append(soff[-1] + Gq[q])                 # within-tile col
    qoff = [TG * soff[q] for q in range(NQ + 1)]      # within-run col
    CR = TG * S
    NCOL = GPC * CR
    NSLOT = NCOL * P
    # msg column for within-tile col s of tile ti (static, used by device)
    msg_off = [[qoff[q] + ti * Gq[q] + (s - soff[q])
                for q in range(NQ) for s in range(soff[q], soff[q + 1])]
               for ti in range(TG)]

    per_core = []
    for k in range(NCORES):
        idx16 = np.zeros((16, NSLOT // 16), np.int16)
        dstl = np.full((P, NCOL), 255, np.uint8)
        for r in range(GPC):
            for q in range(NQ):
                if Gq[q] == 0:
                    continue
                NI = TG * Gq[q] * P
                lin = np.zeros(NI, np.int16)
                for ti in range(TG):
                    t = r * TG + ti
                    rel, dl = buckets[(k, t, q)]
                    n = len(rel)
                    lin[ti * Gq[q] * P: ti * Gq[q] * P + n] = rel.astype(np.int16)
                    for pos in range(n):
                        j, p = pos // P, pos % P
                        dstl[p, r * CR + ti * S + soff[q] + j] = dl[pos]
                w = lin.reshape(NI // 16, 16).T
                c0 = (r * CR + qoff[q]) * 8
                idx16[:, c0: c0 + NI // 16] = w
        per_core.append(dict(idx16=idx16, dstl=dstl))

    def to_slot_layout(vals_per_node, pad_value, k):
        out = np.full(S_pad, pad_value, np.float32)
        for g in range(k * GPC, (k + 1) * GPC):
            a, b = graph_start[g], graph_start[g + 1]
            slot = g % GPC
            out[slot * TG * P: slot * TG * P + (b - a)] = vals_per_node[a:b]
        return out.reshape(T, P).T.copy()

    for k in range(NCORES):
        pc = per_core[k]
        pc["dis_own"] = to_slot_layout(dis, 0.0, k)
        pc["padmask"] = to_slot_layout(np.ones(N, np.float32), 0.0, k)
        pc["invc"] = (1.0 / np.maximum(gsizes[k * GPC:(k + 1) * GPC], 1)
                      ).astype(np.float32).reshape(1, GPC)

    return dict(
        T=T, TG=TG, S_pad=S_pad, TBL=TBL, NQ=NQ, QROWS=QROWS,
        Gq=Gq, S=S, soff=soff, qoff=qoff, CR=CR, NCOL=NCOL, NSLOT=NSLOT,
        msg_off=msg_off, graph_start=graph_start, map_row=map_row,
        gsizes=gsizes,
    ), per_core


def pack_xT16(x, meta, core):
    """x -> transposed fp16 layout [D_IN, S_pad]: col t*P+p = x[node(t,p)]."""
    S_pad = meta["S_pad"]
    map_row = meta["map_row"]
    xp = np.zeros((meta["TBL"], D_IN), np.float16)
    xp[map_row] = x.astype(np.float16)
    xp = xp[core * S_pad:(core + 1) * S_pad]
    return np.ascontiguousarray(xp.T)


def pack_wcst(inputs, meta, pc):
    """fp16 [128, 192 + 2T]: weights block + per-core dis / padmask."""
    T = meta["T"]
    Wp = np.zeros((128, 192 + 2 * T), np.float16)
    Wp[:, 0:64] = np.asarray(inputs["W_pool"], np.float32)
    Wp[0:64, 64:128] = np.asarray(inputs["W_g1"], np.float32)
    Wp[64:128, 64:128] = np.asarray(inputs["W_g2"], np.float32)
    Wp[0:8, 128:192] = np.asarray(inputs["W_emb"], np.float32)
    Wp[8, 128:192] = np.asarray(inputs["b_emb"], np.float32)
    Wp[9, 128:192] = np.asarray(inputs["b_g1"], np.float32)
    Wp[10, 128:192] = np.asarray(inputs["b_g2"], np.float32)
    Wp[11, 128:192] = np.asarray(inputs["b_pool"], np.float32)
    Wp[12, 128:130] = np.asarray(inputs["b_cls"], np.float32)
    Wp[64:128, 130:132] = np.asarray(inputs["W_cls"], np.float32)
    Wp[13, 128:128 + GPC] = pc["invc"][0]
    Wp[:, 192:192 + T] = pc["dis_own"]
    Wp[:, 192 + T:192 + 2 * T] = pc["padmask"]
    return Wp


# ----------------------------------------------------------------------------
# device program
# ----------------------------------------------------------------------------

def build_program(meta, stage=5, debug_dump=False):
    import concourse.mybir as mybir
    import concourse.tile as tile
    from concourse import bacc
    from concourse.bass import ds
    from concourse.masks import make_identity

    f32 = mybir.dt.float32
    f16 = mybir.dt.float16
    i16 = mybir.dt.int16
    u8 = mybir.dt.uint8
    i32 = mybir.dt.int32
    AF = mybir.ActivationFunctionType
    ALU = mybir.AluOpType
    AX = mybir.AxisListType

    T, TG, S_pad, TBL, NQ, QROWS = (meta[k] for k in
        ["T", "TG", "S_pad", "TBL", "NQ", "QROWS"])
    Gq, S, soff, qoff, CR, NCOL, NSLOT, msg_off = (meta[k] for k in
        ["Gq", "S", "soff", "qoff", "CR", "NCOL", "NSLOT", "msg_off"])
    WB = 8  # tiles per prologue batch (one PSUM bank: 8*64=512 f32)
    assert T % WB == 0

    nc = bacc.Bacc("TRN2", target_bir_lowering=False)

    xT_d = nc.dram_tensor("xT16", [D_IN, S_pad], f16, kind="ExternalInput")
    idx_d = nc.dram_tensor("idx16", [16, NSLOT // 16], i16, kind="ExternalInput")
    dstl_d = nc.dram_tensor("dstl", [P, NCOL], u8, kind="ExternalInput")
    wc_d = nc.dram_tensor("wcst", [128, 192 + 2 * T], f16, kind="ExternalInput")
    out_d = nc.dram_tensor("out", [GPC, C], f32, kind="ExternalOutput")
    dump_d = (nc.dram_tensor("dump", [P, T * H], f32, kind="ExternalOutput")
              if debug_dump else None)
    dumpm_d = (nc.dram_tensor("dumpm", [P, CR * H], f32, kind="ExternalOutput")
               if debug_dump in ("msg", "agg") else None)
    dumpi_d = (nc.dram_tensor("dumpi", [P, CR * 8], i16, kind="ExternalOutput")
               if debug_dump == "msg" else None)

    u0_shard = nc.dram_tensor("u0_shard", [S_pad, H], f32)
    u0_tab = nc.dram_tensor("u0_tab", [TBL, H], f32)
    u1_shard = nc.dram_tensor("u1_shard", [S_pad, H], f32)
    u1_tab = nc.dram_tensor("u1_tab", [TBL, H], f32)

    # primed view: [P, T*H] (partition p, tile-major contiguous)
    def primed(tensor):
        return tensor[:, :].rearrange("(p c) f -> p (c f)", p=P)

    with tile.TileContext(nc) as tc:
        with (
            tc.tile_pool(name="const", bufs=1) as cp,
            tc.tile_pool(name="mpool", bufs=1) as mp,
            tc.tile_pool(name="sbuf", bufs=2) as sp,
            tc.tile_pool(name="psum", bufs=2, space="PSUM") as pp,
        ):
            # ---------------- constants
            ident = cp.tile([P, P], f32)
            make_identity(nc, ident[:])
            iota_i = cp.tile([P, P], i32)
            nc.gpsimd.iota(iota_i[:], pattern=[[1, P]], base=0, channel_multiplier=0)
            iota_f = cp.tile([P, P], f32)
            nc.vector.tensor_copy(iota_f[:], iota_i[:])
            ones_row = cp.tile([1, P], f32)
            nc.gpsimd.memset(ones_row[:], 1.0)
            ones_col = cp.tile([P, 1], f32)
            nc.gpsimd.memset(ones_col[:], 1.0)

            wc16 = cp.tile([128, 192 + 2 * T], f16)
            nc.sync.dma_start(wc16[:], wc_d[:])
            W_emb16 = wc16[0:D_IN, 128:192]
            dp32 = cp.tile([P, 2 * T], f32)  # dis | padmask
            nc.vector.tensor_copy(dp32[:], wc16[:, 192:192 + 2 * T])
            dis_own = dp32[:, 0:T]
            padmask_t = dp32[:, T:2 * T]

            def load_f32(shape, src_ap, tag):
                t16 = sp.tile(shape, f16, tag=f"{tag}_16")
                nc.sync.dma_start(t16[:], src_ap)
                t32 = cp.tile(shape, f32, tag=tag)
                nc.vector.tensor_copy(t32[:], t16[:])
                return t32

            W_pool = load_f32([2 * H, H], wc_d[:, 0:64], "W_pool")
            W_g1 = load_f32([H, H], wc_d[0:H, 64:128], "W_g1")
            W_g2 = load_f32([H, H], wc_d[H:2 * H, 64:128], "W_g2")
            W_cls = load_f32([H, C], wc_d[H:2 * H, 130:132], "W_cls")
            invc_t = load_f32([1, GPC], wc_d[13:14, 128:128 + GPC], "invc")

            b_bcast = {}
            for nm, row in [("emb", 8), ("g1", 9), ("g2", 10)]:
                br = load_f32([1, H], wc_d[row:row + 1, 128:192], f"brow_{nm}")
                ps_b = pp.tile([P, H], f32, tag="ps_b", space="PSUM")
                nc.tensor.matmul(ps_b[:], lhsT=ones_row[:], rhs=br[:],
                                 start=True, stop=True)
                bb = cp.tile([P, H], f32, tag=f"bb_{nm}")
                nc.vector.tensor_copy(bb[:], ps_b[:])
                b_bcast[nm] = bb
            b_pool_r = load_f32([1, H], wc_d[11:12, 128:192], "b_pool_r")
            ps_bp = pp.tile([H, 1], f32, tag="ps_b", space="PSUM")
            nc.tensor.transpose(ps_bp[:], b_pool_r[:], ident[0:1, 0:1])
            b_pool_c = cp.tile([H, 1], f32)
            nc.vector.tensor_copy(b_pool_c[:], ps_bp[:])
            b_cls_r = load_f32([1, C], wc_d[12:13, 128:130], "b_cls_r")
            ps_bc = pp.tile([C, 1], f32, tag="ps_b", space="PSUM")
            nc.tensor.transpose(ps_bc[:], b_cls_r[:], ident[0:1, 0:1])
            b_cls_c = cp.tile([C, 1], f32)
            nc.vector.tensor_copy(b_cls_c[:], ps_bc[:])

            # gather indices: load [16, X], replicate to 128 partitions
            idx_all = cp.tile([P, NSLOT // 16], i16)
            nc.sync.dma_start(idx_all[0:16, :], idx_d[:])
            nc.sync.dma_start(idx_all[16:32, :], idx_all[0:16, :])
            nc.sync.dma_start(idx_all[32:64, :], idx_all[0:32, :])
            nc.sync.dma_start(idx_all[64:128, :], idx_all[0:64, :])

            # dst one-hot source: u8 -> f32 once
            dsl_u8 = cp.tile([P, NCOL], u8)
            nc.sync.dma_start(dsl_u8[:], dstl_d[:])
            dsl_all = cp.tile([P, NCOL], f32)
            nc.vector.tensor_copy(dsl_all[:], dsl_u8[:])

            # persistent slabs
            u0slab = cp.tile([P, T * H], f32)
            u1slab = cp.tile([P, T * H], f32)
            maxT = cp.tile([H, GPC], f32)
            sumT = cp.tile([H, GPC], f32)

            # ---------------- prologue: u0 for own shard (fp16 matmul)
            with tc.For_i(0, T // WB, 1) as ib:
                xsl = sp.tile([D_IN, WB * P], f16, tag="xsl")
                nc.sync.dma_start(xsl[:], xT_d[:, ds(ib * (WB * P), WB * P)])
                ps_slab = pp.tile([P, WB * H], f32, tag="ps_a", space="PSUM")
                for i in range(WB):
                    nc.tensor.matmul(
                        ps_slab[:, i * H:(i + 1) * H],
                        lhsT=xsl[:, i * P:(i + 1) * P],
                        rhs=W_emb16[:],
                        start=True, stop=True)
                s_sl = sp.tile([P, WB * H], f32, tag="s_pro")
                nc.vector.tensor_tensor(
                    out=s_sl[:].rearrange("p (t f) -> p t f", f=H),
                    in0=ps_slab[:].rearrange("p (t f) -> p t f", f=H),
                    in1=b_bcast["emb"][:].unsqueeze(1).to_broadcast([P, WB, H]),
                    op=ALU.add)
                r_sl = sp.tile([P, WB * H], f32, tag="r_pro")
                nc.scalar.activation(r_sl[:], s_sl[:], AF.Relu)
                nc.vector.tensor_tensor(
                    out=u0slab[:, ds(ib * (WB * H), WB * H)].rearrange(
                        "p (t f) -> p t f", f=H),
                    in0=r_sl[:].rearrange("p (t f) -> p t f", f=H),
                    in1=dp32[:, ds(ib * WB, WB)].unsqueeze(2)
                        .to_broadcast([P, WB, H]),
                    op=ALU.mult)
            nc.sync.dma_start(primed(u0_shard)[:, :], u0slab[:])

            def early_out(src_dram):
                tmp = sp.tile([GPC, C], f32, tag="eo")
                nc.sync.dma_start(tmp[:], src_dram[0:GPC, 0:C])
                nc.sync.dma_start(out_d[:], tmp[:])

            # ---------------- conv layers (nested hw loops: run x tile)
            def conv(table, uslab, W_L, bb_L, last):
                if last:
                    nc.gpsimd.memset(sumT[:], 0.0)
                    nc.gpsimd.memset(maxT[:], 0.0)
                with tc.For_i(0, GPC, 1) as ir:
                    idx_stg = mp.tile([P, CR * 8], i16, tag="idx_stg")
                    nc.vector.tensor_copy(
                        idx_stg[:], idx_all[:, ds(ir * (CR * 8), CR * 8)])
                    msg = mp.tile([P, CR * H], f32, tag="msg")
                    for q in range(NQ):
                        if Gq[q] == 0:
                            continue
                        NI = TG * Gq[q] * P
                        nrows = min(QROWS, TBL - q * QROWS)
                        nc.gpsimd.dma_gather(
                            out_ap=msg[:, qoff[q] * H:(qoff[q + 1]) * H]
                                .rearrange("p (g f) -> p g f", f=H),
                            in_ap=table[q * QROWS: q * QROWS + nrows, :],
                            idxs_ap=idx_stg[:, qoff[q] * 8:qoff[q + 1] * 8],
                            num_idxs=NI, num_idxs_reg=NI, elem_size=H,
                            single_packet=False)
                    with tc.For_i(0, TG, 1) as ti:
                        ps_agg = pp.tile([P, H], f32, tag="ps_a", space="PSUM")
                        nc.tensor.matmul(
                            ps_agg[:], lhsT=ident[:],
                            rhs=uslab[:, ds(ir * (TG * H) + ti * H, H)],
                            start=True, stop=False)
                        M_t = mp.tile([P, S * P], f32, tag="M_t")
                        nc.vector.tensor_tensor(
                            out=M_t[:].rearrange("p (s q) -> p s q", q=P),
                            in0=dsl_all[:, ds(ir * CR + ti * S, S)].unsqueeze(2)
                                .to_broadcast([P, S, P]),
                            in1=iota_f[:].unsqueeze(1).to_broadcast([P, S, P]),
                            op=ALU.is_equal)
                        for s in range(S):
                            q = next(qq for qq in range(NQ)
                                     if soff[qq] <= s < soff[qq + 1])
                            j = s - soff[q]
                            nc.tensor.matmul(
                                ps_agg[:],
                                lhsT=M_t[:, s * P:(s + 1) * P],
                                rhs=msg[:, ds(ti * (Gq[q] * H)
                                              + (qoff[q] + j) * H, H)],
                                start=False, stop=(s == S - 1))
                        v_t = sp.tile([P, H], f32, tag="v_t")
                        nc.vector.tensor_tensor(
                            out=v_t[:], in0=ps_agg[:],
                            in1=dp32[:, ds(ir * TG + ti, 1)]
                                .to_broadcast([P, H]),
                            op=ALU.mult)
                        ps_vt = pp.tile([H, P], f32, tag="ps_b", space="PSUM")
                        nc.tensor.transpose(ps_vt[:], v_t[:], ident[:])
                        vt_s = sp.tile([H, P], f32, tag="vt_s")
                        nc.vector.tensor_copy(vt_s[:], ps_vt[:])
                        ps_o = pp.tile([P, H], f32, tag="ps_o", space="PSUM")
                        nc.tensor.matmul(ps_o[:], lhsT=vt_s[:], rhs=W_L[:],
                                         start=True, stop=True)
                        s2 = sp.tile([P, H], f32, tag="s2")
                        nc.vector.tensor_tensor(out=s2[:], in0=ps_o[:],
                                                in1=bb_L[:], op=ALU.add)
                        r2 = sp.tile([P, H], f32, tag="r2")
                        nc.scalar.activation(r2[:], s2[:], AF.Relu)
                        if not last:
                            nc.vector.tensor_tensor(
                                out=u1slab[:, ds(ir * (TG * H) + ti * H, H)],
                                in0=r2[:],
                                in1=dp32[:, ds(ir * TG + ti, 1)]
                                    .to_broadcast([P, H]),
                                op=ALU.mult)
                        else:
                            h2 = sp.tile([P, H], f32, tag="h2")
                            nc.vector.tensor_tensor(
                                out=h2[:], in0=r2[:],
                                in1=dp32[:, ds(T + ir * TG + ti, 1)]
                                    .to_broadcast([P, H]),
                                op=ALU.mult)
                            ps_h2t = pp.tile([H, P], f32, tag="ps_b",
                                             space="PSUM")
                            nc.tensor.transpose(ps_h2t[:], h2[:], ident[:])
                            h2t_s = sp.tile([H, P], f32, tag="h2t_s")
                            nc.vector.tensor_copy(h2t_s[:], ps_h2t[:])
                            red = sp.tile([H, 1], f32, tag="red")
                            nc.vector.reduce_max(red[:], h2t_s[:], axis=AX.X)
                            nc.vector.tensor_tensor(
                                out=maxT[:, ds(ir, 1)],
                                in0=maxT[:, ds(ir, 1)],
                                in1=red[:], op=ALU.max)
                            reds = sp.tile([H, 1], f32, tag="reds")
                            nc.vector.reduce_sum(reds[:], h2t_s[:], axis=AX.X)
                            nc.vector.tensor_tensor(
                                out=sumT[:, ds(ir, 1)],
                                in0=sumT[:, ds(ir, 1)],
                                in1=reds[:], op=ALU.add)

            if stage == 1:
                early_out(u0_tab)
            if stage >= 2:
                nc.gpsimd.collective_compute(
                    "AllGather", ALU.bypass,
                    replica_groups=[list(range(NCORES))],
                    ins=[u0_shard[:]], outs=[u0_tab[:]])
                if stage == 2:
                    early_out(u0_tab)
            if stage >= 3:
                conv(u0_tab, u0slab, W_g1, b_bcast["g1"], last=False)
                nc.sync.dma_start(primed(u1_shard)[:, :], u1slab[:])
                if debug_dump:
                    nc.sync.dma_start(dump_d[:, :], u1slab[:])
                if stage == 3:
                    early_out(u1_shard)
            if stage >= 4:
                nc.gpsimd.collective_compute(
                    "AllGather", ALU.bypass,
                    replica_groups=[list(range(NCORES))],
                    ins=[u1_shard[:]], outs=[u1_tab[:]])
                if stage == 4:
                    early_out(u1_tab)
            if stage >= 5:
                conv(u1_tab, u1slab, W_g2, b_bcast["g2"], last=True)

                # ---------------- head
                ps_ic = pp.tile([H, GPC], f32, tag="ps_b", space="PSUM")
                nc.tensor.matmul(ps_ic[:], lhsT=ones_row[:, :H], rhs=invc_t[:],
                                 start=True, stop=True)
                ic_s = sp.tile([H, GPC], f32, tag="ic_s")
                nc.vector.tensor_copy(ic_s[:], ps_ic[:])
                meanT = sp.tile([H, GPC], f32, tag="meanT")
                nc.vector.tensor_tensor(out=meanT[:], in0=sumT[:], in1=ic_s[:],
                                        op=ALU.mult)
                cat_s = sp.tile([P, GPC], f32, tag="cat_s")
                nc.sync.dma_start(cat_s[0:H, :], meanT[:])
                nc.sync.dma_start(cat_s[H:2 * H, :], maxT[:])
                ps_hg = pp.tile([H, GPC], f32, tag="ps_b", space="PSUM")
                nc.tensor.matmul(ps_hg[:], lhsT=W_pool[:], rhs=cat_s[:],
                                 start=True, stop=True)
                hg_s = sp.tile([H, GPC], f32, tag="hg_s")
                nc.vector.tensor_tensor(out=hg_s[:], in0=ps_hg[:],
                                        in1=b_pool_c[:].to_broadcast([H, GPC]),
                                        op=ALU.add)
                ps_lg = pp.tile([C, GPC], f32, tag="ps_b", space="PSUM")
                nc.tensor.matmul(ps_lg[:], lhsT=W_cls[:], rhs=hg_s[:],
                                 start=True, stop=True)
                lg_s = sp.tile([C, GPC], f32, tag="lg_s")
                nc.vector.tensor_tensor(out=lg_s[:], in0=ps_lg[:],
                                        in1=b_cls_c[:].to_broadcast([C, GPC]),
                                        op=ALU.add)
                ps_z = pp.tile([GPC, C], f32, tag="ps_b", space="PSUM")
                nc.tensor.transpose(ps_z[:], lg_s[:], ident[0:C, 0:C])
                z = sp.tile([GPC, C], f32, tag="z")
                nc.vector.tensor_copy(z[:], ps_z[:])
                zm = sp.tile([GPC, 1], f32, tag="zm")
                nc.vector.reduce_max(zm[:], z[:], axis=AX.X)
                zs = sp.tile([GPC, C], f32, tag="zs")
                nc.vector.tensor_tensor(out=zs[:], in0=z[:],
                                        in1=zm[:].to_broadcast([GPC, C]),
                                        op=ALU.subtract)
                ez = sp.tile([GPC, C], f32, tag="ez")
                nc.scalar.activation(ez[:], zs[:], AF.Exp)
                es = sp.tile([GPC, 1], f32, tag="es")
                nc.vector.reduce_sum(es[:], ez[:], axis=AX.X)
                les = sp.tile([GPC, 1], f32, tag="les")
                nc.scalar.activation(les[:], es[:], AF.Ln)
                res = sp.tile([GPC, C], f32, tag="res")
                nc.vector.tensor_tensor(out=res[:], in0=zs[:],
                                        in1=les[:].to_broadcast([GPC, C]),
                                        op=ALU.subtract)
                nc.sync.dma_start(out_d[:], res[:])

    nc.finalize()
    return nc


# ----------------------------------------------------------------------------
# entry point
# ----------------------------------------------------------------------------

_trace = {"on": False, "res": None}


def kernel(**inputs):
    from concourse.bass_utils import run_bass_kernel_spmd

    x = np.asarray(inputs["x"], np.float32)
    src = np.asarray(inputs["src"])
    dst = np.asarray(inputs["dst"])
    batch = np.asarray(inputs["batch"])

    meta, per_core = build_meta(src, dst, batch)

    in_maps = []
    for k in range(NCORES):
        pc = per_core[k]
        in_maps.append(dict(
            xT16=pack_xT16(x, meta, k),
            idx16=pc["idx16"],
            dstl=pc["dstl"],
            wcst=pack_wcst(inputs, meta, pc),
        ))

    nc = build_program(meta, stage=_trace.get("stage", 5))
    _trace["nc"] = nc
    _trace["in_maps"] = in_maps
    res = run_bass_kernel_spmd(
        nc, in_maps, core_ids=list(range(NCORES)),
        trace=_trace["on"])
    _trace["res"] = res
    out = np.concatenate([res.results[k]["out"] for k in range(NCORES)], axis=0)
    return out.astype(np.float32)


# revision 4
# speedup vs baseline: 1.0968x; 1.0968x over previous
"""Trainium2 Bass kernel for nn_CascadeGNN (2-layer GCN + mean/max pool + cls).

v3 — optimized for the axon PJRT per-call cost model: measured per-call time
= RPC floor + c1 * NEFF-size(~instruction count) + c2 * input-bytes, with
actual device exec being tiny. So this version:
  - uses a UNIFORM edge schedule (per-quarter column count Gq shared by all
    tiles/cores) so both conv layers are a single hardware loop (tc.For_i)
    over the 16 graph-slots -> ~10x fewer instructions than full unrolling;
  - runs are graph-aligned (RUN == TG), which kills the batch one-hot
    (mean-pool accumulates straight into a per-run PSUM column) and the
    separate batch/count inputs;
  - ships ~1.2 MB per core: fp16 xT, unreplicated int16 gather indices,
    uint8 dst rows, and one packed fp16 weight/constant tensor;
  - computes u0/u1 on the own shard and AllGathers the u tables on-device.

Math identity: with u = dis * h, a GCN layer is
    h' = relu(dis * (sum_{e: src->n} u[src] + u[n]) @ W + b)
so cores exchange only u tables and apply W post-aggregation.  Edge messages
are fetched with SWDGE dma_gather (int16 indices over <=32767-row table
quarters) and segment-summed on the TensorEngine via one-hot matrices built
on the VectorEngine from the uint8 dst-row table; PSUM accumulates; the self
term is an identity matmul from the SBUF-resident own-u slab.  Pad nodes are
zeroed via dis=0 / padmask.

The Bass program is compiled per input instance (edge schedule baked in).
"""
import numpy as np

P = 128
NCORES = 8
H = 64
D_IN = 8
GPC = 16

N = 100000
E = 1600000
G = 128
C = 2


# ----------------------------------------------------------------------------
# host-side metadata (sharding / index prep)
# ----------------------------------------------------------------------------

def build_meta(src, dst, batch):
    graph_start = np.searchsorted(batch, np.arange(G + 1))
    gsizes = (graph_start[1:] - graph_start[:-1]).astype(np.int64)
    TG = int(np.ceil(max(int(gsizes.max()), 1) / P))
    T = GPC * TG
    S_pad = T * P
    TBL = NCORES * S_pad
    NQ = int(np.ceil(TBL / 32767.0))
    QROWS = int(np.ceil(TBL / NQ / P)) * P

    # node -> padded table row (logical: local = tile*128 + partition)
    map_row = np.empty(N, np.int64)
    for g in range(G):
        k, slot = g // GPC, g % GPC
        a, b = graph_start[g], graph_start[g + 1]
        map_row[a:b] = k * S_pad + slot * TG * P + np.arange(b - a)

    deg = np.bincount(dst, minlength=N).astype(np.float64) + 1.0
    dis = (1.0 / np.sqrt(deg)).astype(np.float32)

    order = np.argsort(dst, kind="stable")
    src_s = src[order].astype(np.int64)
    dst_s = dst[order].astype(np.int64)
    # primed (partition-major) table row of the source
    sr = map_row[src_s]
    sk, sloc = sr // S_pad, sr % S_pad
    src_rowp = sk * S_pad + (sloc % P) * T + (sloc // P)
    src_q = src_rowp // QROWS
    src_rel = (src_rowp - src_q * QROWS).astype(np.int64)
    dst_row = map_row[dst_s]

    buckets = {}
    cnt = np.zeros((NCORES, T, NQ), np.int64)
    for k in range(NCORES):
        e0 = np.searchsorted(dst_row, k * S_pad)
        e1 = np.searchsorted(dst_row, (k + 1) * S_pad)
        loc = dst_row[e0:e1] - k * S_pad
        tq = loc // P
        t_start = e0 + np.searchsorted(tq, np.arange(T + 1))
        for t in range(T):
            a, b = t_start[t], t_start[t + 1]
            q_e = src_q[a:b]
            loc_t = loc[a - e0:b - e0] - t * P
            for q in range(NQ):
                m = q_e == q
                buckets[(k, t, q)] = (src_rel[a:b][m], loc_t[m])
                cnt[k, t, q] = int(m.sum())

    # uniform schedule: per-quarter column count shared by all tiles/cores
    Gq = [int(x) for x in (-(-cnt // P)).max(axis=(0, 1))]
    S = int(sum(Gq))
    soff = [0]
    for q in range(NQ):
        soff.append(soff[-1] + Gq[q])                 # within-tile col
    qoff = [TG * soff[q] for q in range(NQ + 1)]      # within-run col
    CR = TG * S
    NCOL = GPC * CR
    NSLOT = NCOL * P
    # msg column for within-tile col s of tile ti (static, used by device)
    msg_off = [[qoff[q] + ti * Gq[q] + (s - soff[q])
                for q in range(NQ) for s in range(soff[q], soff[q + 1])]
               for ti in range(TG)]

    per_core = []
    for k in range(NCORES):
        idx16 = np.zeros((16, NSLOT // 16), np.int16)
        dstl = np.full((P, NCOL), 255, np.uint8)
        for r in range(GPC):
            for q in range(NQ):
                if Gq[q] == 0:
                    continue
                NI = TG * Gq[q] * P
                lin = np.zeros(NI, np.int16)
                for ti in range(TG):
                    t = r * TG + ti
                    rel, dl = buckets[(k, t, q)]
                    n = len(rel)
                    lin[ti * Gq[q] * P: ti * Gq[q] * P + n] = rel.astype(np.int16)
                    for pos in range(n):
                        j, p = pos // P, pos % P
                        dstl[p, r * CR + ti * S + soff[q] + j] = dl[pos]
                w = lin.reshape(NI // 16, 16).T
                c0 = (r * CR + qoff[q]) * 8
                idx16[:, c0: c0 + NI // 16] = w
        per_core.append(dict(idx16=idx16, dstl=dstl))

    def to_slot_layout(vals_per_node, pad_value, k):
        out = np.full(S_pad, pad_value, np.float32)
        for g in range(k * GPC, (k + 1) * GPC):
            a, b = graph_start[g], graph_start[g + 1]
            slot = g % GPC
            out[slot * TG * P: slot * TG * P + (b - a)] = vals_per_node[a:b]
        return out.reshape(T, P).T.copy()

    for k in range(NCORES):
        pc = per_core[k]
        pc["dis_own"] = to_slot_layout(dis, 0.0, k)
        pc["padmask"] = to_slot_layout(np.ones(N, np.float32), 0.0, k)
        pc["invc"] = (1.0 / np.maximum(gsizes[k * GPC:(k + 1) * GPC], 1)
                      ).astype(np.float32).reshape(1, GPC)

    return dict(
        T=T, TG=TG, S_pad=S_pad, TBL=TBL, NQ=NQ, QROWS=QROWS,
        Gq=Gq, S=S, soff=soff, qoff=qoff, CR=CR, NCOL=NCOL, NSLOT=NSLOT,
        msg_off=msg_off, graph_start=graph_start, map_row=map_row,
        gsizes=gsizes,
    ), per_core


def pack_xT16(x, meta, core):
    """x -> transposed fp16 layout [D_IN, S_pad]: col t*P+p = x[node(t,p)]."""
    S_pad = meta["S_pad"]
    map_row = meta["map_row"]
    xp = np.zeros((meta["TBL"], D_IN), np.float16)
    xp[map_row] = x.astype(np.float16)
    xp = xp[core * S_pad:(core + 1) * S_pad]
    return np.ascontiguousarray(xp.T)


def pack_wcst(inputs, meta, pc):
    """fp16 [128, 128 + 2T]: conv weights block + per-core dis / padmask.

    (Pool/cls head weights stay on the host - the head runs in numpy.)
    """
    T = meta["T"]
    Wp = np.zeros((128, 128 + 2 * T), np.float16)
    Wp[0:64, 0:64] = np.asarray(inputs["W_g1"], np.float32)
    Wp[64:128, 0:64] = np.asarray(inputs["W_g2"], np.float32)
    Wp[0:8, 64:128] = np.asarray(inputs["W_emb"], np.float32)
    Wp[8, 64:128] = np.asarray(inputs["b_emb"], np.float32)
    Wp[9, 64:128] = np.asarray(inputs["b_g1"], np.float32)
    Wp[10, 64:128] = np.asarray(inputs["b_g2"], np.float32)
    Wp[:, 128:128 + T] = pc["dis_own"]
    Wp[:, 128 + T:128 + 2 * T] = pc["padmask"]
    return Wp


# ----------------------------------------------------------------------------
# device program
# ----------------------------------------------------------------------------

def build_program(meta, stage=5, debug_dump=False):
    import concourse.mybir as mybir
    import concourse.tile as tile
    from concourse import bacc
    from concourse.bass import ds
    from concourse.masks import make_identity

    f32 = mybir.dt.float32
    f16 = mybir.dt.float16
    i16 = mybir.dt.int16
    u8 = mybir.dt.uint8
    i32 = mybir.dt.int32
    AF = mybir.ActivationFunctionType
    ALU = mybir.AluOpType
    AX = mybir.AxisListType

    T, TG, S_pad, TBL, NQ, QROWS = (meta[k] for k in
        ["T", "TG", "S_pad", "TBL", "NQ", "QROWS"])
    Gq, S, soff, qoff, CR, NCOL, NSLOT, msg_off = (meta[k] for k in
        ["Gq", "S", "soff", "qoff", "CR", "NCOL", "NSLOT", "msg_off"])
    WB = 8  # tiles per prologue batch (one PSUM bank: 8*64=512 f32)
    assert T % WB == 0

    nc = bacc.Bacc("TRN2", target_bir_lowering=False)

    xT_d = nc.dram_tensor("xT16", [D_IN, S_pad], f16, kind="ExternalInput")
    idx_d = nc.dram_tensor("idx16", [16, NSLOT // 16], i16, kind="ExternalInput")
    dstl_d = nc.dram_tensor("dstl", [P, NCOL], u8, kind="ExternalInput")
    wc_d = nc.dram_tensor("wcst", [128, 128 + 2 * T], f16, kind="ExternalInput")
    out_d = nc.dram_tensor("out", [H, 2 * GPC], f32, kind="ExternalOutput")
    dump_d = (nc.dram_tensor("dump", [P, T * H], f32, kind="ExternalOutput")
              if debug_dump else None)
    dumpm_d = (nc.dram_tensor("dumpm", [P, CR * H], f32, kind="ExternalOutput")
               if debug_dump in ("msg", "agg") else None)
    dumpi_d = (nc.dram_tensor("dumpi", [P, CR * 8], i16, kind="ExternalOutput")
               if debug_dump == "msg" else None)

    u0_shard = nc.dram_tensor("u0_shard", [S_pad, H], f32)
    u0_tab = nc.dram_tensor("u0_tab", [TBL, H], f32)
    u1_shard = nc.dram_tensor("u1_shard", [S_pad, H], f32)
    u1_tab = nc.dram_tensor("u1_tab", [TBL, H], f32)

    # primed view: [P, T*H] (partition p, tile-major contiguous)
    def primed(tensor):
        return tensor[:, :].rearrange("(p c) f -> p (c f)", p=P)

    with tile.TileContext(nc) as tc:
        with (
            tc.tile_pool(name="const", bufs=1) as cp,
            tc.tile_pool(name="mpool", bufs=1) as mp,
            tc.tile_pool(name="sbuf", bufs=2) as sp,
            tc.tile_pool(name="psum", bufs=2, space="PSUM") as pp,
        ):
            # ---------------- constants
            ident = cp.tile([P, P], f32)
            make_identity(nc, ident[:])
            iota_i = cp.tile([P, P], i32)
            nc.gpsimd.iota(iota_i[:], pattern=[[1, P]], base=0, channel_multiplier=0)
            iota_f = cp.tile([P, P], f32)
            nc.vector.tensor_copy(iota_f[:], iota_i[:])
            ones_row = cp.tile([1, P], f32)
            nc.gpsimd.memset(ones_row[:], 1.0)

            wc16 = cp.tile([128, 128 + 2 * T], f16)
            nc.sync.dma_start(wc16[:], wc_d[:])
            W_emb16 = wc16[0:D_IN, 64:128]
            dp32 = cp.tile([P, 2 * T], f32)  # dis | padmask
            nc.vector.tensor_copy(dp32[:], wc16[:, 128:128 + 2 * T])
            dis_own = dp32[:, 0:T]

            def load_f32(shape, src_ap, tag):
                t16 = sp.tile(shape, f16, tag=f"{tag}_16")
                nc.sync.dma_start(t16[:], src_ap)
                t32 = cp.tile(shape, f32, tag=tag)
                nc.vector.tensor_copy(t32[:], t16[:])
                return t32

            W_g1 = load_f32([H, H], wc_d[0:H, 0:64], "W_g1")
            W_g2 = load_f32([H, H], wc_d[H:2 * H, 0:64], "W_g2")

            b_bcast = {}
            for nm, row in [("emb", 8), ("g1", 9), ("g2", 10)]:
                br = load_f32([1, H], wc_d[row:row + 1, 64:128], f"brow_{nm}")
                ps_b = pp.tile([P, H], f32, tag="ps_b", space="PSUM")
                nc.tensor.matmul(ps_b[:], lhsT=ones_row[:], rhs=br[:],
                                 start=True, stop=True)
                bb = cp.tile([P, H], f32, tag=f"bb_{nm}")
                nc.vector.tensor_copy(bb[:], ps_b[:])
                b_bcast[nm] = bb

            # gather indices: load [16, X], replicate to 128 partitions
            idx_all = cp.tile([P, NSLOT // 16], i16)
            nc.sync.dma_start(idx_all[0:16, :], idx_d[:])
            nc.sync.dma_start(idx_all[16:32, :], idx_all[0:16, :])
            nc.sync.dma_start(idx_all[32:64, :], idx_all[0:32, :])
            nc.sync.dma_start(idx_all[64:128, :], idx_all[0:64, :])

            # dst one-hot source: u8 -> f32 once
            dsl_u8 = cp.tile([P, NCOL], u8)
            nc.sync.dma_start(dsl_u8[:], dstl_d[:])
            dsl_all = cp.tile([P, NCOL], f32)
            nc.vector.tensor_copy(dsl_all[:], dsl_u8[:])

            # persistent slabs
            u0slab = cp.tile([P, T * H], f32)
            u1slab = cp.tile([P, T * H], f32)
            maxT = cp.tile([H, GPC], f32)
            sumT = cp.tile([H, GPC], f32)

            # ---------------- prologue: u0 for own shard (fp16 matmul)
            with tc.For_i(0, T // WB, 1) as ib:
                xsl = sp.tile([D_IN, WB * P], f16, tag="xsl")
                nc.sync.dma_start(xsl[:], xT_d[:, ds(ib * (WB * P), WB * P)])
                ps_slab = pp.tile([P, WB * H], f32, tag="ps_a", space="PSUM")
                for i in range(WB):
                    nc.tensor.matmul(
                        ps_slab[:, i * H:(i + 1) * H],
                        lhsT=xsl[:, i * P:(i + 1) * P],
                        rhs=W_emb16[:],
                        start=True, stop=True)
                s_sl = sp.tile([P, WB * H], f32, tag="s_pro")
                nc.vector.tensor_tensor(
                    out=s_sl[:].rearrange("p (t f) -> p t f", f=H),
                    in0=ps_slab[:].rearrange("p (t f) -> p t f", f=H),
                    in1=b_bcast["emb"][:].unsqueeze(1).to_broadcast([P, WB, H]),
                    op=ALU.add)
                r_sl = sp.tile([P, WB * H], f32, tag="r_pro")
                nc.scalar.activation(r_sl[:], s_sl[:], AF.Relu)
                nc.vector.tensor_tensor(
                    out=u0slab[:, ds(ib * (WB * H), WB * H)].rearrange(
                        "p (t f) -> p t f", f=H),
                    in0=r_sl[:].rearrange("p (t f) -> p t f", f=H),
                    in1=dp32[:, ds(ib * WB, WB)].unsqueeze(2)
                        .to_broadcast([P, WB, H]),
                    op=ALU.mult)
            nc.sync.dma_start(primed(u0_shard)[:, :], u0slab[:])

            def early_out(src_dram):
                tmp = sp.tile([GPC, C], f32, tag="eo")
                nc.sync.dma_start(tmp[:], src_dram[0:GPC, 0:C])
                nc.sync.dma_start(out_d[0:GPC, 0:C], tmp[:])

            # ---------------- conv layers (nested hw loops: run x tile)
            def conv(table, uslab, W_L, bb_L, last):
                if last:
                    nc.gpsimd.memset(sumT[:], 0.0)
                    nc.gpsimd.memset(maxT[:], 0.0)
                with tc.For_i(0, GPC, 1) as ir:
                    idx_stg = mp.tile([P, CR * 8], i16, tag="idx_stg")
                    nc.vector.tensor_copy(
                        idx_stg[:], idx_all[:, ds(ir * (CR * 8), CR * 8)])
                    msg = mp.tile([P, CR * H], f32, tag="msg")
                    for q in range(NQ):
                        if Gq[q] == 0:
                            continue
                        NI = TG * Gq[q] * P
                        nrows = min(QROWS, TBL - q * QROWS)
                        nc.gpsimd.dma_gather(
                            out_ap=msg[:, qoff[q] * H:(qoff[q + 1]) * H]
                                .rearrange("p (g f) -> p g f", f=H),
                            in_ap=table[q * QROWS: q * QROWS + nrows, :],
                            idxs_ap=idx_stg[:, qoff[q] * 8:qoff[q + 1] * 8],
                            num_idxs=NI, num_idxs_reg=NI, elem_size=H,
                            single_packet=False)
                    with tc.For_i(0, TG, 1) as ti:
                        ps_agg = pp.tile([P, H], f32, tag="ps_a", space="PSUM")
                        nc.tensor.matmul(
                            ps_agg[:], lhsT=ident[:],
                            rhs=uslab[:, ds(ir * (TG * H) + ti * H, H)],
                            start=True, stop=False)
                        M_t = mp.tile([P, S * P], f32, tag="M_t")
                        nc.vector.tensor_tensor(
                            out=M_t[:].rearrange("p (s q) -> p s q", q=P),
                            in0=dsl_all[:, ds(ir * CR + ti * S, S)].unsqueeze(2)
                                .to_broadcast([P, S, P]),
                            in1=iota_f[:].unsqueeze(1).to_broadcast([P, S, P]),
                            op=ALU.is_equal)
                        for s in range(S):
                            q = next(qq for qq in range(NQ)
                                     if soff[qq] <= s < soff[qq + 1])
                            j = s - soff[q]
                            nc.tensor.matmul(
                                ps_agg[:],
                                lhsT=M_t[:, s * P:(s + 1) * P],
                                rhs=msg[:, ds(ti * (Gq[q] * H)
                                              + (qoff[q] + j) * H, H)],
                                start=False, stop=(s == S - 1))
                        v_t = sp.tile([P, H], f32, tag="v_t")
                        nc.vector.tensor_tensor(
                            out=v_t[:], in0=ps_agg[:],
                            in1=dp32[:, ds(ir * TG + ti, 1)]
                                .to_broadcast([P, H]),
                            op=ALU.mult)
                        ps_vt = pp.tile([H, P], f32, tag="ps_b", space="PSUM")
                        nc.tensor.transpose(ps_vt[:], v_t[:], ident[:])
                        vt_s = sp.tile([H, P], f32, tag="vt_s")
                        nc.vector.tensor_copy(vt_s[:], ps_vt[:])
                        ps_o = pp.tile([P, H], f32, tag="ps_o", space="PSUM")
                        nc.tensor.matmul(ps_o[:], lhsT=vt_s[:], rhs=W_L[:],
                                         start=True, stop=True)
                        s2 = sp.tile([P, H], f32, tag="s2")
                        nc.vector.tensor_tensor(out=s2[:], in0=ps_o[:],
                                                in1=bb_L[:], op=ALU.add)
                        r2 = sp.tile([P, H], f32, tag="r2")
                        nc.scalar.activation(r2[:], s2[:], AF.Relu)
                        if not last:
                            nc.vector.tensor_tensor(
                                out=u1slab[:, ds(ir * (TG * H) + ti * H, H)],
                                in0=r2[:],
                                in1=dp32[:, ds(ir * TG + ti, 1)]
                                    .to_broadcast([P, H]),
                                op=ALU.mult)
                        else:
                            h2 = sp.tile([P, H], f32, tag="h2")
                            nc.vector.tensor_tensor(
                                out=h2[:], in0=r2[:],
                                in1=dp32[:, ds(T + ir * TG + ti, 1)]
                                    .to_broadcast([P, H]),
                                op=ALU.mult)
                            ps_h2t = pp.tile([H, P], f32, tag="ps_b",
                                             space="PSUM")
                            nc.tensor.transpose(ps_h2t[:], h2[:], ident[:])
                            h2t_s = sp.tile([H, P], f32, tag="h2t_s")
                            nc.vector.tensor_copy(h2t_s[:], ps_h2t[:])
                            red = sp.tile([H, 1], f32, tag="red")
                            nc.vector.reduce_max(red[:], h2t_s[:], axis=AX.X)
                            nc.vector.tensor_tensor(
                                out=maxT[:, ds(ir, 1)],
                                in0=maxT[:, ds(ir, 1)],
                                in1=red[:], op=ALU.max)
                            reds = sp.tile([H, 1], f32, tag="reds")
                            nc.vector.reduce_sum(reds[:], h2t_s[:], axis=AX.X)
                            nc.vector.tensor_tensor(
                                out=sumT[:, ds(ir, 1)],
                                in0=sumT[:, ds(ir, 1)],
                                in1=reds[:], op=ALU.add)

            if stage == 1:
                early_out(u0_tab)
            if stage >= 2:
                nc.gpsimd.collective_compute(
                    "AllGather", ALU.bypass,
                    replica_groups=[list(range(NCORES))],
                    ins=[u0_shard[:]], outs=[u0_tab[:]])
                if stage == 2:
                    early_out(u0_tab)
            if stage >= 3:
                conv(u0_tab, u0slab, W_g1, b_bcast["g1"], last=False)
                nc.sync.dma_start(primed(u1_shard)[:, :], u1slab[:])
                if debug_dump:
                    nc.sync.dma_start(dump_d[:, :], u1slab[:])
                if stage == 3:
                    early_out(u1_shard)
            if stage >= 4:
                nc.gpsimd.collective_compute(
                    "AllGather", ALU.bypass,
                    replica_groups=[list(range(NCORES))],
                    ins=[u1_shard[:]], outs=[u1_tab[:]])
                if stage == 4:
                    early_out(u1_tab)
            if stage >= 5:
                conv(u1_tab, u1slab, W_g2, b_bcast["g2"], last=True)

                # pooled sums/maxes go back to the host (head runs in numpy)
                nc.sync.dma_start(out_d[:, 0:GPC], sumT[:])
                nc.sync.dma_start(out_d[:, GPC:2 * GPC], maxT[:])

    nc.finalize()
    return nc


# ----------------------------------------------------------------------------
# entry point
# ----------------------------------------------------------------------------

_trace = {"on": False, "res": None}


def kernel(**inputs):
    from concourse.bass_utils import run_bass_kernel_spmd

    x = np.asarray(inputs["x"], np.float32)
    src = np.asarray(inputs["src"])
    dst = np.asarray(inputs["dst"])
    batch = np.asarray(inputs["batch"])

    meta, per_core = build_meta(src, dst, batch)

    in_maps = []
    for k in range(NCORES):
        pc = per_core[k]
        in_maps.append(dict(
            xT16=pack_xT16(x, meta, k),
            idx16=pc["idx16"],
            dstl=pc["dstl"],
            wcst=pack_wcst(inputs, meta, pc),
        ))

    nc = build_program(meta, stage=_trace.get("stage", 5))
    _trace["nc"] = nc
    _trace["in_maps"] = in_maps
    _trace["per_core"] = per_core
    _trace["inputs"] = inputs
    res = run_bass_kernel_spmd(
        nc, in_maps, core_ids=list(range(NCORES)),
        trace=_trace["on"])
    _trace["res"] = res
    return head_host(res.results, per_core, inputs)


def head_host(results, per_core, inputs):
    W_pool = np.asarray(inputs["W_pool"], np.float32)
    b_pool = np.asarray(inputs["b_pool"], np.float32)
    W_cls = np.asarray(inputs["W_cls"], np.float32)
    b_cls = np.asarray(inputs["b_cls"], np.float32)
    outs = []
    for k in range(NCORES):
        sm = np.asarray(results[k]["out"], np.float32)   # [H, 2*GPC]
        mean = sm[:, 0:GPC] * per_core[k]["invc"]        # [H, GPC]
        cat = np.concatenate([mean, sm[:, GPC:2 * GPC]], axis=0)  # [2H, GPC]
        hg = cat.T @ W_pool + b_pool                     # [GPC, H]
        z = hg @ W_cls + b_cls                           # [GPC, C]
        z = z - z.max(axis=1, keepdims=True)
        lse = np.log(np.exp(z).sum(axis=1, keepdims=True))
        outs.append((z - lse).astype(np.float32))
    return np.concatenate(outs, axis=0)
